# revision 45
# baseline (speedup 1.0000x reference)
"""Hypernetwork causal attention (nn_Attention_87926570484382) on 8 TRN2 cores.

Strategy (two launches, batch-sharded attention, gen-sharded hypernet).
Baseline 468us -> ~290us.

  host   : time-embedding MLP -> t [128]; bias assembly; q-scale folding.
  launch1 (~91us, DMA-bound): W-gen with fW blocks STATIONARY and t as a
           1-column moving operand: each [128,128] block -> one dense psum
           column, so psum->sbuf copies shrink 128x vs the replicated-t
           formulation. The q,k slab streams as fp8e4m3 (prescaled x16,
           t x32; softmax path tolerates it: rel err 1.2e-3 vs 4.4e-4),
           v/proj slabs in f16. 25.2MB/core at ~420GB/s.
  host   : gather W slices (free), add biases, fold 1/sqrt(D) into q.
  launch2 (~199us): attention, 2 batches/core, organized for PE density
           and a thrash-free scalar engine:
           - scores of each head PAIR row-packed (K=64 x 2 row groups,
             concurrent in the array via auto tile_position)
           - ONE exp per (pair, j, qc): both heads' psum banks in one
             contiguous [128, 512+w] activation (gap exp'd harmlessly)
           - AV with [v|ones] interleaved stationary: one accumulation
             group per bank (rows 0-63 sum exp*v, 64-127 denominators)
           - per-batch reciprocal CLUSTERS on the scalar engine (raw
             InstActivation; exp and reciprocal live in different ACT
             tables, so interleaving costs 1.28us/switch -- add_dep_helper
             edges pin the ACT order to [exps][recips][exps][recips])
           - qkv(b+1)/proj(b-1) blocks woven into attention pair slots
             where PSUM is free (j=4,6), rest bunched at the batch
             boundary to cover the reciprocal drain; keeps HAM warm
"""

import os
import sys

import numpy as np

# ---------------------------------------------------------------------------
# Environment shims (must precede concourse imports in fresh environments)
# ---------------------------------------------------------------------------


def _ensure_axon_hooks():
    """Provide antenv.axon_hooks if the installed antenv lacks it (needed
    only when tracing; harmless otherwise)."""
    try:
        import antenv.axon_hooks  # noqa: F401
        return
    except ImportError:
        pass
    try:
        import antenv
    except ImportError:
        return
    import contextlib
    import ctypes
    import types

    mod = types.ModuleType("antenv.axon_hooks")
    mod._HOOK = None
    mod._TRIED = False

    def set_axon_ntff_profile_hook(hook):
        mod._HOOK = hook

    def _build(so_path):
        lib = ctypes.CDLL(so_path)
        if not hasattr(lib, "axon_start_nrt_profile"):
            return None
        lib.axon_start_nrt_profile.argtypes = [
            ctypes.POINTER(ctypes.c_int64),
            ctypes.c_size_t,
        ]
        lib.axon_start_nrt_profile.restype = ctypes.c_int64
        lib.axon_stop_nrt_profile.argtypes = [ctypes.c_char_p]
        lib.axon_stop_nrt_profile.restype = ctypes.c_int64

        @contextlib.contextmanager
        def _hook(output_dir, device_ids):
            import jax

            jax.devices()
            if device_ids:
                ids = (ctypes.c_int64 * len(device_ids))(*device_ids)
                rc = lib.axon_start_nrt_profile(ids, len(device_ids))
            else:
                rc = lib.axon_start_nrt_profile(None, 0)
            if rc != 0:
                raise RuntimeError(f"axon_start_nrt_profile rc={rc}")
            try:
                yield
            finally:
                n = lib.axon_stop_nrt_profile(str(output_dir).encode())
                print(f"profile: {n} file(s) -> {output_dir}", file=sys.stderr)

        return _hook

    def get_axon_ntff_profile_hook():
        if mod._HOOK is None and not mod._TRIED:
            mod._TRIED = True
            p = "/opt/axon/libaxon_pjrt.so"
            if os.path.exists(p):
                try:
                    mod._HOOK = _build(p)
                except OSError:
                    mod._HOOK = None
        return mod._HOOK

    mod.set_axon_ntff_profile_hook = set_axon_ntff_profile_hook
    mod.get_axon_ntff_profile_hook = get_axon_ntff_profile_hook
    sys.modules["antenv.axon_hooks"] = mod
    antenv.axon_hooks = mod


_ensure_axon_hooks()

import concourse.bass as bass  # noqa: E402
import concourse.mybir as mybir  # noqa: E402
from concourse import tile as _tile  # noqa: E402
from concourse.tile import TileContext, add_dep_helper  # noqa: E402
from concourse.vector_clock import ScopedClock  # noqa: E402
from concourse.bass_utils import run_bass_kernel_spmd  # noqa: E402

F32 = mybir.dt.float32
F16 = mybir.dt.float16
F8 = mybir.dt.float8e4

# problem constants (hardcoded per harness contract)
SIN_DIM, TEMBED = 64, 128
E, H, D = 512, 8, 64
B, S = 16, 1024
NCORES = 8
BPC = B // NCORES          # batches per core
S2 = BPC * S               # 2048 rows per core
J3 = 3 * H * D             # 1536
EJ_A = E * J3 // NCORES    # 98304  fW_attn slab cols per core
EJ_P = E * E // NCORES     # 32768  fW_proj slab cols per core

# ---------------------------------------------------------------------------
# Tile framework workarounds: this walrus accepts at most ONE semaphore wait
# and one update per instruction.
# ---------------------------------------------------------------------------

_NOP_CTR = [0]


def _patched_drain_and_barrier(self, tick_clock, wait_clock):
    carrier = self.nc.sync.nop(nofuse=True)
    wait_clock.add_sem_waits(
        carrier.ins, ScopedClock({None: tick_clock.global_clock})
    )
    si = carrier.ins.sync_info
    waits = list(si.on_wait) if si and si.on_wait else []
    if len(waits) > 1:
        carrier.ins.sync_info = mybir.SyncInfo(
            on_wait=waits[:1],
            on_update=list(si.on_update) if si and si.on_update else [],
        )
        for w in waits[1:]:
            extra = self.nc.sync.nop(nofuse=True)
            extra.ins.sync_info = mybir.SyncInfo(on_wait=[w], on_update=[])
    self.nc.sync.drain()
    self.nc.all_engine_barrier()
    assert self.sems is not None
    popped = self.nc._tile_sem_poison_stack.pop()
    assert popped is self._sem_poison
    self.nc.clear_and_free_semaphores(list(self.sems.allocated().values()))
    self.nc.all_engine_barrier()


_tile.TileContext._drain_and_barrier = _patched_drain_and_barrier


def _split_multi_waits(nc):
    for f in nc.m.functions:
        for blk in f.blocks:
            out = []
            changed = False
            for inst in blk.instructions:
                si = inst.sync_info
                waits = list(si.on_wait) if si and si.on_wait else []
                updates = list(si.on_update) if si and si.on_update else []
                is_dma = "DMA" in type(inst).__name__
                if len(waits) > 1:
                    changed = True
                    for w in waits[:-1]:
                        _NOP_CTR[0] += 1
                        nop = mybir.InstNoOp(
                            name=f"wsplit_{_NOP_CTR[0]}", ins=[], outs=[]
                        )
                        nop.engine = inst.engine
                        nop.sync_info = mybir.SyncInfo(on_wait=[w], on_update=[])
                        out.append(nop)
                    waits = [waits[-1]]
                    inst.sync_info = mybir.SyncInfo(
                        on_wait=waits, on_update=updates
                    )
                out.append(inst)
                if len(updates) > 1:
                    if is_dma:
                        raise AssertionError(
                            f"DMA {inst.name} has {len(updates)} updates"
                        )
                    changed = True
                    inst.sync_info = mybir.SyncInfo(
                        on_wait=waits, on_update=[updates[0]]
                    )
                    for u in updates[1:]:
                        _NOP_CTR[0] += 1
                        nop = mybir.InstNoOp(
                            name=f"usplit_{_NOP_CTR[0]}", ins=[], outs=[]
                        )
                        nop.engine = inst.engine
                        nop.sync_info = mybir.SyncInfo(on_wait=[], on_update=[u])
                        out.append(nop)
            if changed:
                blk.instructions = out
    return nc


# ---------------------------------------------------------------------------
# Launch 1: hypernetwork weight generation
# ---------------------------------------------------------------------------


EJ_Q = E * 1024 // NCORES  # 65536  q,k cols per core (fp8)
EJ_V = E * 512 // NCORES   # 32768  v cols per core (f16)


def build_gen():
    """Per core: W[c] = sum_k t[k] * fW_slab[k, c].  Each [128,128] fW block
    is the STATIONARY operand; t streams as a single moving column, so each
    block yields one dense psum column (128 W values on 128 partitions).
    The q,k slab streams in fp8 (softmax path tolerates it), v/proj in f16.
    Output layout: g*[p, i] = W[128*i + p] (host transposes)."""
    nc = bass.Bass()
    tcol = nc.dram_tensor("tcol", [TEMBED, 1], F16, kind="ExternalInput")
    tcol8 = nc.dram_tensor("tcol8", [TEMBED, 1], F8, kind="ExternalInput")
    fq = nc.dram_tensor("fq", [TEMBED, EJ_Q], F8, kind="ExternalInput")
    fv = nc.dram_tensor("fv", [TEMBED, EJ_V], F16, kind="ExternalInput")
    fwp = nc.dram_tensor("fwp", [TEMBED, EJ_P], F16, kind="ExternalInput")
    gq = nc.dram_tensor("gq", [128, EJ_Q // 128], F32, kind="ExternalOutput")
    gv = nc.dram_tensor("gv", [128, EJ_V // 128], F32, kind="ExternalOutput")
    gp = nc.dram_tensor("gp", [128, EJ_P // 128], F32, kind="ExternalOutput")

    LOAD = 8192  # dma granularity in columns

    with TileContext(nc) as tc:
        with (
            tc.tile_pool(name="cst", bufs=1) as cst,
            tc.tile_pool(name="sb", bufs=5) as sb,
            tc.tile_pool(name="ev", bufs=4) as ev,
            tc.tile_pool(name="ps", bufs=4, space="PSUM") as ps,
        ):
            tt = cst.tile([TEMBED, 1], F16)
            nc.sync.dma_start(out=tt, in_=tcol[:, :])
            tt8 = cst.tile([TEMBED, 1], F8)
            nc.sync.dma_start(out=tt8, in_=tcol8[:, :])

            cnt = 0
            slabs = (
                (fq, gq, EJ_Q, F8),
                (fv, gv, EJ_V, F16),
                (fwp, gp, EJ_P, F16),
            )
            for si, (src, dst, total, dt) in enumerate(slabs):
                mv = tt8 if dt == F8 else tt
                chunks = [LOAD] * (total // LOAD)
                if si == len(slabs) - 1:
                    # taper the final chunks so the post-DMA serial tail
                    # (matmuls+copy+store of the last chunk) shrinks
                    chunks = chunks[:-1] + [4096, 2048, 1024, 1024]
                lo = 0
                for sz in chunks:
                    nb = sz // 128
                    ft = sb.tile([TEMBED, LOAD], dt, tag=f"fw{dt}")
                    nc.sync.dma_start(
                        out=ft[:, 0:sz], in_=src[:, lo : lo + sz]
                    )
                    pt = ps.tile([128, 64], F32, tag="ps")
                    for blk in range(nb):
                        # each column is its own group: start=True only
                        # clears has_written bits, data of other columns
                        # persists, so this is order-independent
                        nc.tensor.matmul(
                            pt[:, blk : blk + 1],
                            ft[:, 128 * blk : 128 * (blk + 1)],
                            mv,
                            start=True,
                            stop=True,
                        )
                    gs = ev.tile([128, 64], F32, tag="gs")
                    if cnt % 2 == 0:
                        nc.vector.tensor_copy(gs[:, 0:nb], pt[:, 0:nb])
                    else:
                        nc.scalar.copy(gs[:, 0:nb], pt[:, 0:nb])
                    col = lo // 128
                    nc.gpsimd.dma_start(
                        out=dst[:, col : col + nb], in_=gs[:, 0:nb]
                    )
                    cnt += 1
                    lo += sz
    _split_multi_waits(nc)
    return nc


# ---------------------------------------------------------------------------
# Launch 2: attention for 2 batches per core
# ---------------------------------------------------------------------------


def _act_recip(nc, out, in_):
    """Reciprocal on the scalar engine (raw InstActivation; the python
    wrapper rejects Reciprocal, but its ~1e-3 accuracy is ample for
    softmax denominators and it is ~6x faster than DVE InstReciprocal)."""
    eng = nc.scalar
    imm = lambda v: mybir.ImmediateValue(dtype=mybir.dt.float32, value=v)
    return eng.add_instruction(
        mybir.InstActivation(
            name=eng.bass.get_next_instruction_name(),
            func=mybir.ActivationFunctionType.Reciprocal,
            ins=[eng.lower_ap(in_), imm(0.0), imm(1.0), imm(0.0)],
            outs=[eng.lower_ap(out)],
        )
    )


def build_attn():
    nc = bass.Bass()
    xt = nc.dram_tensor("xt", [E, S2], F16, kind="ExternalInput")
    wa = nc.dram_tensor("wa", [E, J3], F16, kind="ExternalInput")
    wp = nc.dram_tensor("wp", [E, E], F16, kind="ExternalInput")
    bqk = nc.dram_tensor("bqk", [128, 8], F32, kind="ExternalInput")
    brow = nc.dram_tensor("brow", [1, E], F16, kind="ExternalInput")
    mask = nc.dram_tensor("mask", [128, 128], F16, kind="ExternalInput")
    ones = nc.dram_tensor("ones", [128, 128], F16, kind="ExternalInput")
    out = nc.dram_tensor("out", [S2, E], F32, kind="ExternalOutput")

    NQT = S // 128           # 8 q/k tiles per batch
    NET = E // 128           # 4 e tiles

    with TileContext(nc) as tc:
        with (
            tc.tile_pool(name="cst", bufs=1) as cst,
            tc.tile_pool(name="qk", bufs=1) as qkp,
            tc.tile_pool(name="vx", bufs=1) as vxp,
            tc.tile_pool(name="ot", bufs=1) as otp,
            tc.tile_pool(name="ex", bufs=6) as exp_pool,
            tc.tile_pool(name="os", bufs=2) as osp,
            tc.tile_pool(name="y0", bufs=4) as y0p,
            tc.tile_pool(name="ob", bufs=3) as obp,
            tc.tile_pool(name="ps", bufs=4, space="PSUM") as ps,
        ):
            # resident inputs, DMA'd in need-order at fine granularity so
            # the first qkv matmul starts after ~1.2MB instead of ~4.2MB
            bqk_t = cst.tile([128, 8], F32)
            nc.sync.dma_start(out=bqk_t, in_=bqk[:, :])
            xts = [cst.tile([128, S2], F16, tag=f"xt{et}", name=f"xt{et}")
                   for et in range(NET)]
            was = [cst.tile([128, J3], F16, tag=f"wa{et}", name=f"wa{et}")
                   for et in range(NET)]
            wps = [cst.tile([128, E], F16, tag=f"wp{et}", name=f"wp{et}")
                   for et in range(NET)]
            for et in range(NET):  # batch-0 half of x^T
                nc.sync.dma_start(
                    out=xts[et][:, 0:S],
                    in_=xt[128 * et : 128 * (et + 1), 0:S],
                )
            for et in range(NET):
                nc.sync.dma_start(
                    out=was[et], in_=wa[128 * et : 128 * (et + 1), :]
                )
            mask_t = cst.tile([128, 128], F16)
            nc.sync.dma_start(out=mask_t, in_=mask[:, :])
            for et in range(NET):  # batch-1 half of x^T
                nc.sync.dma_start(
                    out=xts[et][:, S:S2],
                    in_=xt[128 * et : 128 * (et + 1), S:S2],
                )
            for et in range(NET):
                nc.sync.dma_start(
                    out=wps[et], in_=wp[128 * et : 128 * (et + 1), :]
                )
            brow_t = cst.tile([1, E], F16)
            nc.sync.dma_start(out=brow_t, in_=brow[:, :])
            ones1 = cst.tile([1, 128], F16)
            nc.sync.dma_start(out=ones1, in_=ones[0:1, :])

            # per-batch persistent tiles (double set: batch b+1 generated
            # while batch b consumed)
            qkts = {
                b: [
                    qkp.tile([128, S], F16, tag=f"qk{b}_{m}", name=f"qk{b}_{m}")
                    for m in range(8)
                ]
                for b in range(BPC)
            }
            # v tiles: [128 k-rows, 8h x (64 v | 64 ones)]; ones halves are
            # memset once and persist, v halves rewritten per batch
            vxs = {
                b: [
                    vxp.tile(
                        [128, 1024], F16, tag=f"vx{b}_{st}", name=f"vx{b}_{st}"
                    )
                    for st in range(NQT)
                ]
                for b in range(BPC)
            }
            for b in range(BPC):
                for st in range(NQT):
                    nc.gpsimd.memset(vxs[b][st][:, :], 1.0)

            # ACT-stream ordering state: exp instrs per batch, and the gate
            # (last recip of the previous batch's norm cluster) that all of
            # the next batch's exps must follow -- keeps the scalar engine
            # stream as [exps(0)][recips(0)][exps(1)][recips(1)] so the
            # activation table loads only 4x instead of per-switch
            exp_all = []
            act_gate = [None]
            last_av = [None]
            pe_gate = [None]
            ots = {
                b: [
                    otp.tile(
                        [128, S], F16, tag=f"ot{b}_{ht}", name=f"ot{b}_{ht}"
                    )
                    for ht in range(NET)
                ]
                for b in range(BPC)
            }

            def qkv_block(b, m):
                s0 = b * S
                pq = ps.tile([128, 1024], F32, tag="ps", name=f"pq{b}_{m}")
                for sc in range(2):
                    for et in range(NET):
                        mi = nc.tensor.matmul(
                            pq[:, 512 * sc : 512 * (sc + 1)],
                            was[et][:, 128 * m : 128 * (m + 1)],
                            xts[et][:, s0 + 512 * sc : s0 + 512 * (sc + 1)],
                            start=(et == 0),
                            stop=(et == NET - 1),
                        )
                        if sc == 0 and et == 0 and pe_gate[0] is not None:
                            add_dep_helper(
                                mi.ins, pe_gate[0], reason="boundary filler"
                            )
                nc.vector.tensor_scalar_add(qkts[b][m], pq, bqk_t[:, m : m + 1])

            def v_block(b, st):
                s0 = b * S
                pv = ps.tile([128, 1024], F32, tag="ps", name=f"pv{b}_{st}")
                for et in range(NET):
                    mi = nc.tensor.matmul(
                        pv[:, 0:512],
                        xts[et][:, s0 + 128 * st : s0 + 128 * (st + 1)],
                        was[et][:, 1024:1536],
                        start=(et == 0),
                        stop=(et == NET - 1),
                    )
                    if et == 0 and pe_gate[0] is not None:
                        add_dep_helper(
                            mi.ins, pe_gate[0], reason="boundary filler"
                        )
                # strided copy into the v halves (cols 128h + [0,64))
                dst = vxs[b][st][:, :].rearrange(
                    "p (h t d) -> p h t d", t=2, d=64
                )[:, :, 0:1, :].squeeze()
                src = pv[:, 0:512].rearrange("p (h d) -> p h d", h=8)
                nc.vector.tensor_copy(dst, src)

            def proj_block(b, st):
                s0 = b * S
                pp = ps.tile([128, 1024], F32, tag="ps", name=f"pp{b}_{st}")
                for ht in range(NET):
                    mi = nc.tensor.matmul(
                        pp[:, 0:512],
                        ots[b][ht][:, 128 * st : 128 * (st + 1)],
                        wps[ht],
                        start=(ht == 0),
                        stop=False,
                    )
                    if ht == 0 and pe_gate[0] is not None:
                        add_dep_helper(
                            mi.ins, pe_gate[0], reason="boundary filler"
                        )
                nc.tensor.matmul(
                    pp[:, 0:512], ones1, brow_t, start=False, stop=True
                )
                ob = obp.tile([128, 512], F32, tag="ob", name=f"ob{b}_{st}")
                nc.vector.tensor_copy(ob, pp[:, 0:512])
                nc.sync.dma_start(
                    out=out[s0 + 128 * st : s0 + 128 * (st + 1), :], in_=ob
                )

            def scores_exp(b, hp, j):
                """Row-packed scores for head pair hp at k-tile j, one exp
                per (j, qc) over both heads via a strided [128,2,w] AP.
                Returns {qc: expt tile} with head A at cols [0,w), head B
                at [512, 512+w)."""
                qt = qkts[b][hp]
                kt = qkts[b][4 + hp]
                expts = {}
                for qc in range(2):
                    c0 = max(512 * qc, 128 * j)
                    c1 = 512 * (qc + 1)
                    if c0 >= c1:
                        continue
                    w = c1 - c0
                    pst = ps.tile(
                        [128, 1024], F32, tag="ps", name=f"pst{b}_{hp}_{j}_{qc}"
                    )
                    nc.tensor.matmul(
                        pst[:, 0:w],
                        kt[0:64, 128 * j : 128 * (j + 1)],
                        qt[0:64, c0:c1],
                        start=True,
                        stop=True,
                    )
                    nc.tensor.matmul(
                        pst[:, 512 : 512 + w],
                        kt[64:128, 128 * j : 128 * (j + 1)],
                        qt[64:128, c0:c1],
                        start=True,
                        stop=True,
                    )
                    et = exp_pool.tile(
                        [128, 1024], F16, tag="expt", name=f"ex{b}_{hp}_{j}_{qc}"
                    )
                    # one contiguous exp covering head A [0,w) and head B
                    # [512,512+w); the (unread) gap is exp'd harmlessly
                    ei = nc.scalar.activation(
                        et[:, 0 : 512 + w],
                        pst[:, 0 : 512 + w],
                        func=mybir.ActivationFunctionType.Exp,
                    )
                    exp_all.append(ei.ins)
                    if act_gate[0] is not None:
                        add_dep_helper(
                            ei.ins, act_gate[0], reason="ACT table sandwich"
                        )
                    expts[qc] = et
                # causal mask on the diagonal tile (always the first 128
                # cols of the chunk it lives in)
                etd = expts[j // 4]
                nc.vector.tensor_mul(etd[:, 0:128], etd[:, 0:128], mask_t)
                nc.vector.tensor_mul(
                    etd[:, 512:640], etd[:, 512:640], mask_t
                )
                return expts

            def av(b, hp, j, expts, po):
                """attn@V with [v|ones] interleaved stationary: psum rows
                0-63 = sum exp*v, rows 64-127 = softmax denominators."""
                for qc in range(2):
                    c0 = max(512 * qc, 128 * j)
                    c1 = 512 * (qc + 1)
                    if c0 >= c1:
                        continue
                    w = c1 - c0
                    o0 = c0 - 512 * qc
                    et = expts[qc]
                    last = 3 if qc == 0 else NQT - 1
                    for hi in range(2):
                        h = 2 * hp + hi
                        base = 512 * hi
                        mi = nc.tensor.matmul(
                            po[qc][:, base + o0 : base + o0 + w],
                            vxs[b][j][:, 128 * h : 128 * (h + 1)],
                            et[:, base : base + w],
                            start=(j == 0),
                            stop=(j == last),
                        )
                        last_av[0] = mi.ins

            # staged [v|denom] per (pair, qc); consumed by the per-batch
            # norm cluster (reciprocals batched to avoid ACT table thrash)
            osm = {}

            def attn_pair(b, hp, fillers, every):
                po = {
                    qc: ps.tile(
                        [128, 1024], F32, tag="ps", name=f"po{b}_{hp}_{qc}"
                    )
                    for qc in range(2)
                }
                e = {0: scores_exp(b, hp, 0), 1: scores_exp(b, hp, 1)}
                for j in range(NQT):
                    if j + 2 < NQT:
                        e[j + 2] = scores_exp(b, hp, j + 2)
                    av(b, hp, j, e.pop(j), po)
                    if j == 3:
                        t_ = osp.tile(
                            [128, 1024], F16, tag=f"osm{hp}_0",
                            name=f"osm{b}_{hp}_0",
                        )
                        nc.vector.tensor_copy(t_, po[0])
                        osm[(b, hp, 0)] = t_
                    # fillers only in the second half of the j-loop, where
                    # po[0] is freed and PSUM slots are available
                    if j in (4, 6) and fillers:
                        fillers.pop(0)()
                t_ = osp.tile(
                    [128, 1024], F16, tag=f"osm{hp}_1", name=f"osm{b}_{hp}_1"
                )
                nc.vector.tensor_copy(t_, po[1])
                osm[(b, hp, 1)] = t_

            def norm_cluster(b):
                # recips run contiguously after every exp issued so far
                # (keeps ACT table loads at 2 per cluster); qc-major order
                # lets proj of this batch start after the qc0 muls
                last_exp = exp_all[-1]
                for qc in range(2):
                    for hp in range(4):
                        o_t = osm[(b, hp, qc)]
                        y0 = y0p.tile(
                            [64, 1024], F32, tag="y0", name=f"y0_{b}_{hp}_{qc}"
                        )
                        ri = _act_recip(nc, y0, o_t[64:128, :])
                        add_dep_helper(
                            ri.ins, last_exp, reason="recips after batch exps"
                        )
                        act_gate[0] = ri.ins
                        for hi in range(2):
                            base = 512 * hi
                            prow = 64 * hi
                            nc.vector.tensor_mul(
                                ots[b][hp][
                                    prow : prow + 64, 512 * qc : 512 * (qc + 1)
                                ],
                                o_t[0:64, base : base + 512],
                                y0[:, base : base + 512],
                            )
                        osm.pop((b, hp, qc))

            # ---- schedule ----
            for m in range(8):
                qkv_block(0, m)
            for st in range(NQT):
                v_block(0, st)

            fill1 = [
                (lambda m=m: qkv_block(1, m)) for m in range(8)
            ] + [(lambda st=st: v_block(1, st)) for st in range(NQT)]
            # interleave qkv and v blocks of batch 1
            fill1 = [fill1[i // 2 + 8 * (i % 2)] for i in range(16)]
            # 2 fillers per batch-0 pair (j=4,6); the remaining 8 bunch at
            # the batch boundary where the PE otherwise runs dry waiting
            # for the ACT engine to drain
            for hp in range(4):
                attn_pair(0, hp, fill1, 2)
            # the remaining fillers are PINNED (via dep edges on the last
            # batch-0 AV matmul) to the batch boundary, covering the PE
            # while ACT produces batch-1 pair-0 exps -- otherwise the
            # scheduler hoists them into earlier micro-gaps
            pe_gate[0] = last_av[0]
            while fill1:
                fill1.pop(0)()
            pe_gate[0] = None

            # batch-1 pair 0 runs UNGATED before cluster(0): its exps extend
            # the contiguous exp run, giving the PE AV work while the
            # reciprocal cluster drains on ACT
            attn_pair(1, 0, [], 2)
            norm_cluster(0)

            fill0 = [(lambda st=st: proj_block(0, st)) for st in range(NQT)]
            for hp in range(1, 4):
                attn_pair(1, hp, fill0, 2)
            pe_gate[0] = last_av[0]
            while fill0:
                fill0.pop(0)()
            pe_gate[0] = None
            norm_cluster(1)
            for st in range(NQT):
                proj_block(1, st)
    _split_multi_waits(nc)
    return nc


# ---------------------------------------------------------------------------
# Host orchestration
# ---------------------------------------------------------------------------

_CACHE = {}


def _get(name, builder):
    if name not in _CACHE:
        _CACHE[name] = builder()
    return _CACHE[name]


def _run_with_retry(nc, in_maps, trace=False, tries=3):
    import time as _time

    last = None
    for attempt in range(tries):
        try:
            return run_bass_kernel_spmd(
                nc, in_maps, core_ids=list(range(NCORES)), trace=trace
            )
        except Exception as e:  # transient NRT_EXEC_UNIT_UNRECOVERABLE etc.
            last = e
            _time.sleep(2.0 * (attempt + 1))
    raise last


def _silu(v):
    return v / (1.0 + np.exp(-v))


def kernel(
    time_embed,
    x,
    lin1_w,
    lin1_b,
    lin2_w,
    lin2_b,
    fW_attn_w,
    fW_attn_b,
    fb_attn,
    fW_proj_w,
    fW_proj_b,
    fb_proj,
    _trace=False,
    _times=None,
):
    f64 = np.float64
    # ---- host: time-embedding MLP ----
    t1 = _silu(time_embed.astype(f64) @ lin1_w.astype(f64) + lin1_b.astype(f64))
    t = t1 @ lin2_w.astype(f64) + lin2_b.astype(f64)   # [128]
    t16 = t.astype(np.float16)
    tcol = np.ascontiguousarray(t16[:, None])

    # ---- launch 1: W generation ----
    import ml_dtypes

    f8 = ml_dtypes.float8_e4m3
    QS_W, QS_T = 16.0, 32.0  # prescales keeping fp8 operands in normal range
    nc_gen = _get("gen", build_gen)
    fwa3 = fW_attn_w.reshape(TEMBED, E, J3)
    fq_all = (fwa3[:, :, :1024].astype(np.float32) * QS_W).astype(f8)
    fv_all = fwa3[:, :, 1024:].astype(np.float16)
    fwp_flat = fW_proj_w.reshape(TEMBED, E * E).astype(np.float16)
    tcol8 = np.ascontiguousarray((t * QS_T).astype(np.float32).astype(f8)[:, None])
    in_maps = []
    for c in range(NCORES):
        in_maps.append(
            {
                "tcol": tcol,
                "tcol8": tcol8,
                "fq": np.ascontiguousarray(
                    fq_all[:, 64 * c : 64 * (c + 1), :]
                ).reshape(TEMBED, EJ_Q),
                "fv": np.ascontiguousarray(
                    fv_all[:, 64 * c : 64 * (c + 1), :]
                ).reshape(TEMBED, EJ_V),
                "fwp": fwp_flat[:, EJ_P * c : EJ_P * (c + 1)],
            }
        )
    res1 = _run_with_retry(nc_gen, in_maps, trace=_trace)
    if _times is not None:
        _times.append(res1.exec_time_ns)

    # g*[p, i] = W[128*i + p]  ->  core slab flat = g*.T.ravel()
    Wq = np.concatenate(
        [res1.results[c]["gq"].T.ravel() for c in range(NCORES)]
    ).reshape(E, 1024) / (QS_W * QS_T)
    Wv = np.concatenate(
        [res1.results[c]["gv"].T.ravel() for c in range(NCORES)]
    ).reshape(E, 512)
    Wa = np.concatenate([Wq, Wv], axis=1)
    Wp = np.concatenate(
        [res1.results[c]["gp"].T.ravel() for c in range(NCORES)]
    ).reshape(E, E)
    Wa = Wa + fW_attn_b.reshape(E, J3)
    Wp = Wp + fW_proj_b.reshape(E, E)
    Wa[:, :512] *= 0.125  # fold 1/sqrt(D) into q columns

    # ---- host: biases ----
    b_attn = (t @ fb_attn.astype(f64).reshape(TEMBED, J3)).astype(np.float32)
    bqk_host = b_attn[:1024].copy()
    bqk_host[:512] *= 0.125
    bqk_in = np.ascontiguousarray(bqk_host.reshape(8, 128).T)
    b_v = b_attn[1024:]
    b_proj = (t @ fb_proj.astype(f64)).astype(np.float32)
    brow = (b_v.astype(f64) @ Wp.astype(f64) + b_proj).astype(np.float16)
    brow_in = np.ascontiguousarray(brow[None, :])
    mask_in = np.triu(np.ones((128, 128), dtype=np.float16))
    ones_in = np.ones((128, 128), dtype=np.float16)
    Wa16 = Wa.astype(np.float16)
    Wp16 = Wp.astype(np.float16)

    # ---- launch 2: attention ----
    nc_attn = _get("attn", build_attn)
    in_maps = []
    for c in range(NCORES):
        xt_c = np.ascontiguousarray(
            x[BPC * c : BPC * (c + 1)].reshape(S2, E).T
        )
        in_maps.append(
            {
                "xt": xt_c.astype(np.float16),
                "wa": Wa16,
                "wp": Wp16,
                "bqk": bqk_in,
                "brow": brow_in,
                "mask": mask_in,
                "ones": ones_in,
            }
        )
    res2 = _run_with_retry(nc_attn, in_maps, trace=_trace)
    if _times is not None:
        _times.append(res2.exec_time_ns)

    out = np.empty((B, S, E), dtype=np.float32)
    for c in range(NCORES):
        out[BPC * c : BPC * (c + 1)] = res2.results[c]["out"].reshape(BPC, S, E)
    return out


# revision 46
# speedup vs baseline: 1.0314x; 1.0314x over previous
"""Hypernetwork causal attention (nn_Attention_87926570484382) on 8 TRN2 cores.

Strategy (two launches, batch-sharded attention, gen-sharded hypernet).
Baseline 468us -> ~290us.

  host   : time-embedding MLP -> t [128]; bias assembly; q-scale folding.
  launch1 (~91us, DMA-bound): W-gen with fW blocks STATIONARY and t as a
           1-column moving operand: each [128,128] block -> one dense psum
           column, so psum->sbuf copies shrink 128x vs the replicated-t
           formulation. The q,k slab streams as fp8e4m3 (prescaled x16,
           t x32; softmax path tolerates it: rel err 1.2e-3 vs 4.4e-4),
           v/proj slabs in f16. 25.2MB/core at ~420GB/s.
  host   : gather W slices (free), add biases, fold 1/sqrt(D) into q.
  launch2 (~199us): attention, 2 batches/core, organized for PE density
           and a thrash-free scalar engine:
           - scores of each head PAIR row-packed (K=64 x 2 row groups,
             concurrent in the array via auto tile_position)
           - ONE exp per (pair, j, qc): both heads' psum banks in one
             contiguous [128, 512+w] activation (gap exp'd harmlessly)
           - AV with [v|ones] interleaved stationary: one accumulation
             group per bank (rows 0-63 sum exp*v, 64-127 denominators)
           - per-batch reciprocal CLUSTERS on the scalar engine (raw
             InstActivation; exp and reciprocal live in different ACT
             tables, so interleaving costs 1.28us/switch -- add_dep_helper
             edges pin the ACT order to [exps][recips][exps][recips])
           - qkv(b+1)/proj(b-1) blocks woven into attention pair slots
             where PSUM is free (j=4,6), rest bunched at the batch
             boundary to cover the reciprocal drain; keeps HAM warm
"""

import os
import sys

import numpy as np

# ---------------------------------------------------------------------------
# Environment shims (must precede concourse imports in fresh environments)
# ---------------------------------------------------------------------------


def _ensure_axon_hooks():
    """Provide antenv.axon_hooks if the installed antenv lacks it (needed
    only when tracing; harmless otherwise)."""
    try:
        import antenv.axon_hooks  # noqa: F401
        return
    except ImportError:
        pass
    try:
        import antenv
    except ImportError:
        return
    import contextlib
    import ctypes
    import types

    mod = types.ModuleType("antenv.axon_hooks")
    mod._HOOK = None
    mod._TRIED = False

    def set_axon_ntff_profile_hook(hook):
        mod._HOOK = hook

    def _build(so_path):
        lib = ctypes.CDLL(so_path)
        if not hasattr(lib, "axon_start_nrt_profile"):
            return None
        lib.axon_start_nrt_profile.argtypes = [
            ctypes.POINTER(ctypes.c_int64),
            ctypes.c_size_t,
        ]
        lib.axon_start_nrt_profile.restype = ctypes.c_int64
        lib.axon_stop_nrt_profile.argtypes = [ctypes.c_char_p]
        lib.axon_stop_nrt_profile.restype = ctypes.c_int64

        @contextlib.contextmanager
        def _hook(output_dir, device_ids):
            import jax

            jax.devices()
            if device_ids:
                ids = (ctypes.c_int64 * len(device_ids))(*device_ids)
                rc = lib.axon_start_nrt_profile(ids, len(device_ids))
            else:
                rc = lib.axon_start_nrt_profile(None, 0)
            if rc != 0:
                raise RuntimeError(f"axon_start_nrt_profile rc={rc}")
            try:
                yield
            finally:
                n = lib.axon_stop_nrt_profile(str(output_dir).encode())
                print(f"profile: {n} file(s) -> {output_dir}", file=sys.stderr)

        return _hook

    def get_axon_ntff_profile_hook():
        if mod._HOOK is None and not mod._TRIED:
            mod._TRIED = True
            p = "/opt/axon/libaxon_pjrt.so"
            if os.path.exists(p):
                try:
                    mod._HOOK = _build(p)
                except OSError:
                    mod._HOOK = None
        return mod._HOOK

    mod.set_axon_ntff_profile_hook = set_axon_ntff_profile_hook
    mod.get_axon_ntff_profile_hook = get_axon_ntff_profile_hook
    sys.modules["antenv.axon_hooks"] = mod
    antenv.axon_hooks = mod


_ensure_axon_hooks()

import concourse.bass as bass  # noqa: E402
import concourse.mybir as mybir  # noqa: E402
from concourse import tile as _tile  # noqa: E402
from concourse.tile import TileContext, add_dep_helper  # noqa: E402
from concourse.vector_clock import ScopedClock  # noqa: E402
from concourse.bass_utils import run_bass_kernel_spmd  # noqa: E402

F32 = mybir.dt.float32
F16 = mybir.dt.float16
F8 = mybir.dt.float8e4

# problem constants (hardcoded per harness contract)
SIN_DIM, TEMBED = 64, 128
E, H, D = 512, 8, 64
B, S = 16, 1024
NCORES = 8
BPC = B // NCORES          # batches per core
S2 = BPC * S               # 2048 rows per core
J3 = 3 * H * D             # 1536
EJ_A = E * J3 // NCORES    # 98304  fW_attn slab cols per core
EJ_P = E * E // NCORES     # 32768  fW_proj slab cols per core

# ---------------------------------------------------------------------------
# Tile framework workarounds: this walrus accepts at most ONE semaphore wait
# and one update per instruction.
# ---------------------------------------------------------------------------

_NOP_CTR = [0]


def _patched_drain_and_barrier(self, tick_clock, wait_clock):
    carrier = self.nc.sync.nop(nofuse=True)
    wait_clock.add_sem_waits(
        carrier.ins, ScopedClock({None: tick_clock.global_clock})
    )
    si = carrier.ins.sync_info
    waits = list(si.on_wait) if si and si.on_wait else []
    if len(waits) > 1:
        carrier.ins.sync_info = mybir.SyncInfo(
            on_wait=waits[:1],
            on_update=list(si.on_update) if si and si.on_update else [],
        )
        for w in waits[1:]:
            extra = self.nc.sync.nop(nofuse=True)
            extra.ins.sync_info = mybir.SyncInfo(on_wait=[w], on_update=[])
    self.nc.sync.drain()
    self.nc.all_engine_barrier()
    assert self.sems is not None
    popped = self.nc._tile_sem_poison_stack.pop()
    assert popped is self._sem_poison
    self.nc.clear_and_free_semaphores(list(self.sems.allocated().values()))
    self.nc.all_engine_barrier()


_tile.TileContext._drain_and_barrier = _patched_drain_and_barrier


def _split_multi_waits(nc):
    for f in nc.m.functions:
        for blk in f.blocks:
            out = []
            changed = False
            for inst in blk.instructions:
                si = inst.sync_info
                waits = list(si.on_wait) if si and si.on_wait else []
                updates = list(si.on_update) if si and si.on_update else []
                is_dma = "DMA" in type(inst).__name__
                if len(waits) > 1:
                    changed = True
                    for w in waits[:-1]:
                        _NOP_CTR[0] += 1
                        nop = mybir.InstNoOp(
                            name=f"wsplit_{_NOP_CTR[0]}", ins=[], outs=[]
                        )
                        nop.engine = inst.engine
                        nop.sync_info = mybir.SyncInfo(on_wait=[w], on_update=[])
                        out.append(nop)
                    waits = [waits[-1]]
                    inst.sync_info = mybir.SyncInfo(
                        on_wait=waits, on_update=updates
                    )
                out.append(inst)
                if len(updates) > 1:
                    if is_dma:
                        raise AssertionError(
                            f"DMA {inst.name} has {len(updates)} updates"
                        )
                    changed = True
                    inst.sync_info = mybir.SyncInfo(
                        on_wait=waits, on_update=[updates[0]]
                    )
                    for u in updates[1:]:
                        _NOP_CTR[0] += 1
                        nop = mybir.InstNoOp(
                            name=f"usplit_{_NOP_CTR[0]}", ins=[], outs=[]
                        )
                        nop.engine = inst.engine
                        nop.sync_info = mybir.SyncInfo(on_wait=[], on_update=[u])
                        out.append(nop)
            if changed:
                blk.instructions = out
    return nc


# ---------------------------------------------------------------------------
# Launch 1: hypernetwork weight generation
# ---------------------------------------------------------------------------


EJ_Q = E * 1024 // NCORES  # 65536  q,k cols per core (fp8)
EJ_V = E * 512 // NCORES   # 32768  v cols per core (f16)


def build_gen():
    """Per core: W[c] = sum_k t[k] * fW_slab[k, c].  Each [128,128] fW block
    is the STATIONARY operand; t streams as a single moving column, so each
    block yields one dense psum column (128 W values on 128 partitions).
    The q,k slab streams in fp8 (softmax path tolerates it), v/proj in f16.
    Output layout: g*[p, i] = W[128*i + p] (host transposes)."""
    nc = bass.Bass()
    tcol = nc.dram_tensor("tcol", [TEMBED, 1], F16, kind="ExternalInput")
    tcol8 = nc.dram_tensor("tcol8", [TEMBED, 1], F8, kind="ExternalInput")
    fq = nc.dram_tensor("fq", [TEMBED, EJ_Q], F8, kind="ExternalInput")
    fv = nc.dram_tensor("fv", [TEMBED, EJ_V], F16, kind="ExternalInput")
    fwp = nc.dram_tensor("fwp", [TEMBED, EJ_P], F16, kind="ExternalInput")
    gq = nc.dram_tensor("gq", [128, EJ_Q // 128], F32, kind="ExternalOutput")
    gv = nc.dram_tensor("gv", [128, EJ_V // 128], F32, kind="ExternalOutput")
    gp = nc.dram_tensor("gp", [128, EJ_P // 128], F32, kind="ExternalOutput")

    LOAD = 8192  # dma granularity in columns

    with TileContext(nc) as tc:
        with (
            tc.tile_pool(name="cst", bufs=1) as cst,
            tc.tile_pool(name="sb", bufs=5) as sb,
            tc.tile_pool(name="ev", bufs=4) as ev,
            tc.tile_pool(name="ps", bufs=4, space="PSUM") as ps,
        ):
            tt = cst.tile([TEMBED, 1], F16)
            nc.sync.dma_start(out=tt, in_=tcol[:, :])
            tt8 = cst.tile([TEMBED, 1], F8)
            nc.sync.dma_start(out=tt8, in_=tcol8[:, :])

            cnt = 0
            slabs = (
                (fq, gq, EJ_Q, F8),
                (fv, gv, EJ_V, F16),
                (fwp, gp, EJ_P, F16),
            )
            for si, (src, dst, total, dt) in enumerate(slabs):
                mv = tt8 if dt == F8 else tt
                chunks = [LOAD] * (total // LOAD)
                lo = 0
                for sz in chunks:
                    nb = sz // 128
                    ft = sb.tile([TEMBED, LOAD], dt, tag=f"fw{dt}")
                    nc.sync.dma_start(
                        out=ft[:, 0:sz], in_=src[:, lo : lo + sz]
                    )
                    pt = ps.tile([128, 64], F32, tag="ps")
                    for blk in range(nb):
                        # each column is its own group: start=True only
                        # clears has_written bits, data of other columns
                        # persists, so this is order-independent
                        nc.tensor.matmul(
                            pt[:, blk : blk + 1],
                            ft[:, 128 * blk : 128 * (blk + 1)],
                            mv,
                            start=True,
                            stop=True,
                        )
                    gs = ev.tile([128, 64], F32, tag="gs")
                    if cnt % 2 == 0:
                        nc.vector.tensor_copy(gs[:, 0:nb], pt[:, 0:nb])
                    else:
                        nc.scalar.copy(gs[:, 0:nb], pt[:, 0:nb])
                    col = lo // 128
                    nc.gpsimd.dma_start(
                        out=dst[:, col : col + nb], in_=gs[:, 0:nb]
                    )
                    cnt += 1
                    lo += sz
    _split_multi_waits(nc)
    return nc


# ---------------------------------------------------------------------------
# Launch 2: attention for 2 batches per core
# ---------------------------------------------------------------------------


def _act_recip(nc, out, in_):
    """Reciprocal on the scalar engine (raw InstActivation; the python
    wrapper rejects Reciprocal, but its ~1e-3 accuracy is ample for
    softmax denominators and it is ~6x faster than DVE InstReciprocal)."""
    eng = nc.scalar
    imm = lambda v: mybir.ImmediateValue(dtype=mybir.dt.float32, value=v)
    return eng.add_instruction(
        mybir.InstActivation(
            name=eng.bass.get_next_instruction_name(),
            func=mybir.ActivationFunctionType.Reciprocal,
            ins=[eng.lower_ap(in_), imm(0.0), imm(1.0), imm(0.0)],
            outs=[eng.lower_ap(out)],
        )
    )


def build_attn():
    nc = bass.Bass()
    xt = nc.dram_tensor("xt", [E, S2], F16, kind="ExternalInput")
    wa = nc.dram_tensor("wa", [E, J3], F16, kind="ExternalInput")
    wp = nc.dram_tensor("wp", [E, E], F16, kind="ExternalInput")
    bqk = nc.dram_tensor("bqk", [128, 8], F32, kind="ExternalInput")
    brow = nc.dram_tensor("brow", [1, E], F16, kind="ExternalInput")
    mask = nc.dram_tensor("mask", [128, 128], F16, kind="ExternalInput")
    ones = nc.dram_tensor("ones", [128, 128], F16, kind="ExternalInput")
    out = nc.dram_tensor("out", [S2, E], F32, kind="ExternalOutput")

    NQT = S // 128           # 8 q/k tiles per batch
    NET = E // 128           # 4 e tiles

    with TileContext(nc) as tc:
        with (
            tc.tile_pool(name="cst", bufs=1) as cst,
            tc.tile_pool(name="qk", bufs=1) as qkp,
            tc.tile_pool(name="vx", bufs=1) as vxp,
            tc.tile_pool(name="ot", bufs=1) as otp,
            tc.tile_pool(name="ex", bufs=6) as exp_pool,
            tc.tile_pool(name="os", bufs=2) as osp,
            tc.tile_pool(name="y0", bufs=4) as y0p,
            tc.tile_pool(name="ob", bufs=3) as obp,
            tc.tile_pool(name="ps", bufs=4, space="PSUM") as ps,
        ):
            # resident inputs, DMA'd in need-order at fine granularity so
            # the first qkv matmul starts after ~1.2MB instead of ~4.2MB
            bqk_t = cst.tile([128, 8], F32)
            nc.sync.dma_start(out=bqk_t, in_=bqk[:, :])
            xts = [cst.tile([128, S2], F16, tag=f"xt{et}", name=f"xt{et}")
                   for et in range(NET)]
            was = [cst.tile([128, J3], F16, tag=f"wa{et}", name=f"wa{et}")
                   for et in range(NET)]
            wps = [cst.tile([128, E], F16, tag=f"wp{et}", name=f"wp{et}")
                   for et in range(NET)]
            for et in range(NET):  # batch-0 half of x^T
                nc.sync.dma_start(
                    out=xts[et][:, 0:S],
                    in_=xt[128 * et : 128 * (et + 1), 0:S],
                )
            for et in range(NET):
                nc.sync.dma_start(
                    out=was[et], in_=wa[128 * et : 128 * (et + 1), :]
                )
            mask_t = cst.tile([128, 128], F16)
            nc.sync.dma_start(out=mask_t, in_=mask[:, :])
            for et in range(NET):  # batch-1 half of x^T
                nc.sync.dma_start(
                    out=xts[et][:, S:S2],
                    in_=xt[128 * et : 128 * (et + 1), S:S2],
                )
            for et in range(NET):
                nc.sync.dma_start(
                    out=wps[et], in_=wp[128 * et : 128 * (et + 1), :]
                )
            brow_t = cst.tile([1, E], F16)
            nc.sync.dma_start(out=brow_t, in_=brow[:, :])
            ones1 = cst.tile([1, 128], F16)
            nc.sync.dma_start(out=ones1, in_=ones[0:1, :])

            # per-batch persistent tiles (double set: batch b+1 generated
            # while batch b consumed)
            qkts = {
                b: [
                    qkp.tile([128, S], F16, tag=f"qk{b}_{m}", name=f"qk{b}_{m}")
                    for m in range(8)
                ]
                for b in range(BPC)
            }
            # v tiles: [128 k-rows, 8h x (64 v | 64 ones)]; ones halves are
            # memset once and persist, v halves rewritten per batch
            vxs = {
                b: [
                    vxp.tile(
                        [128, 1024], F16, tag=f"vx{b}_{st}", name=f"vx{b}_{st}"
                    )
                    for st in range(NQT)
                ]
                for b in range(BPC)
            }
            for b in range(BPC):
                for st in range(NQT):
                    nc.gpsimd.memset(vxs[b][st][:, :], 1.0)

            # ACT-stream ordering state: exp instrs per batch, and the gate
            # (last recip of the previous batch's norm cluster) that all of
            # the next batch's exps must follow -- keeps the scalar engine
            # stream as [exps(0)][recips(0)][exps(1)][recips(1)] so the
            # activation table loads only 4x instead of per-switch
            exp_all = []
            act_gate = [None]
            last_av = [None]
            pe_gate = [None]
            ots = {
                b: [
                    otp.tile(
                        [128, S], F16, tag=f"ot{b}_{ht}", name=f"ot{b}_{ht}"
                    )
                    for ht in range(NET)
                ]
                for b in range(BPC)
            }

            def qkv_block(b, m):
                s0 = b * S
                pq = ps.tile([128, 1024], F32, tag="ps", name=f"pq{b}_{m}")
                for sc in range(2):
                    for et in range(NET):
                        mi = nc.tensor.matmul(
                            pq[:, 512 * sc : 512 * (sc + 1)],
                            was[et][:, 128 * m : 128 * (m + 1)],
                            xts[et][:, s0 + 512 * sc : s0 + 512 * (sc + 1)],
                            start=(et == 0),
                            stop=(et == NET - 1),
                        )
                        if sc == 0 and et == 0 and pe_gate[0] is not None:
                            add_dep_helper(
                                mi.ins, pe_gate[0], reason="boundary filler"
                            )
                nc.vector.tensor_scalar_add(qkts[b][m], pq, bqk_t[:, m : m + 1])

            def v_block(b, st):
                s0 = b * S
                pv = ps.tile([128, 1024], F32, tag="ps", name=f"pv{b}_{st}")
                for et in range(NET):
                    mi = nc.tensor.matmul(
                        pv[:, 0:512],
                        xts[et][:, s0 + 128 * st : s0 + 128 * (st + 1)],
                        was[et][:, 1024:1536],
                        start=(et == 0),
                        stop=(et == NET - 1),
                    )
                    if et == 0 and pe_gate[0] is not None:
                        add_dep_helper(
                            mi.ins, pe_gate[0], reason="boundary filler"
                        )
                # strided copy into the v halves (cols 128h + [0,64))
                dst = vxs[b][st][:, :].rearrange(
                    "p (h t d) -> p h t d", t=2, d=64
                )[:, :, 0:1, :].squeeze()
                src = pv[:, 0:512].rearrange("p (h d) -> p h d", h=8)
                nc.vector.tensor_copy(dst, src)

            def proj_block(b, st):
                s0 = b * S
                pp = ps.tile([128, 1024], F32, tag="ps", name=f"pp{b}_{st}")
                for ht in range(NET):
                    mi = nc.tensor.matmul(
                        pp[:, 0:512],
                        ots[b][ht][:, 128 * st : 128 * (st + 1)],
                        wps[ht],
                        start=(ht == 0),
                        stop=False,
                    )
                    if ht == 0 and pe_gate[0] is not None:
                        add_dep_helper(
                            mi.ins, pe_gate[0], reason="boundary filler"
                        )
                nc.tensor.matmul(
                    pp[:, 0:512], ones1, brow_t, start=False, stop=True
                )
                ob = obp.tile([128, 512], F32, tag="ob", name=f"ob{b}_{st}")
                nc.vector.tensor_copy(ob, pp[:, 0:512])
                nc.sync.dma_start(
                    out=out[s0 + 128 * st : s0 + 128 * (st + 1), :], in_=ob
                )

            def scores_exp(b, hp, j):
                """Row-packed scores for head pair hp at k-tile j, one exp
                per (j, qc) over both heads via a strided [128,2,w] AP.
                Returns {qc: expt tile} with head A at cols [0,w), head B
                at [512, 512+w)."""
                qt = qkts[b][hp]
                kt = qkts[b][4 + hp]
                expts = {}
                for qc in range(2):
                    c0 = max(512 * qc, 128 * j)
                    c1 = 512 * (qc + 1)
                    if c0 >= c1:
                        continue
                    w = c1 - c0
                    pst = ps.tile(
                        [128, 1024], F32, tag="ps", name=f"pst{b}_{hp}_{j}_{qc}"
                    )
                    nc.tensor.matmul(
                        pst[:, 0:w],
                        kt[0:64, 128 * j : 128 * (j + 1)],
                        qt[0:64, c0:c1],
                        start=True,
                        stop=True,
                    )
                    nc.tensor.matmul(
                        pst[:, 512 : 512 + w],
                        kt[64:128, 128 * j : 128 * (j + 1)],
                        qt[64:128, c0:c1],
                        start=True,
                        stop=True,
                    )
                    et = exp_pool.tile(
                        [128, 1024], F16, tag="expt", name=f"ex{b}_{hp}_{j}_{qc}"
                    )
                    # one contiguous exp covering head A [0,w) and head B
                    # [512,512+w); the (unread) gap is exp'd harmlessly
                    ei = nc.scalar.activation(
                        et[:, 0 : 512 + w],
                        pst[:, 0 : 512 + w],
                        func=mybir.ActivationFunctionType.Exp,
                    )
                    exp_all.append(ei.ins)
                    if act_gate[0] is not None:
                        add_dep_helper(
                            ei.ins, act_gate[0], reason="ACT table sandwich"
                        )
                    expts[qc] = et
                # causal mask on the diagonal tile (always the first 128
                # cols of the chunk it lives in)
                etd = expts[j // 4]
                nc.vector.tensor_mul(etd[:, 0:128], etd[:, 0:128], mask_t)
                nc.vector.tensor_mul(
                    etd[:, 512:640], etd[:, 512:640], mask_t
                )
                return expts

            def av(b, hp, j, expts, po):
                """attn@V with [v|ones] interleaved stationary: psum rows
                0-63 = sum exp*v, rows 64-127 = softmax denominators."""
                for qc in range(2):
                    c0 = max(512 * qc, 128 * j)
                    c1 = 512 * (qc + 1)
                    if c0 >= c1:
                        continue
                    w = c1 - c0
                    o0 = c0 - 512 * qc
                    et = expts[qc]
                    last = 3 if qc == 0 else NQT - 1
                    for hi in range(2):
                        h = 2 * hp + hi
                        base = 512 * hi
                        mi = nc.tensor.matmul(
                            po[qc][:, base + o0 : base + o0 + w],
                            vxs[b][j][:, 128 * h : 128 * (h + 1)],
                            et[:, base : base + w],
                            start=(j == 0),
                            stop=(j == last),
                        )
                        last_av[0] = mi.ins

            # staged [v|denom] per (pair, qc); consumed by the per-batch
            # norm cluster (reciprocals batched to avoid ACT table thrash)
            osm = {}

            def attn_pair(b, hp, fillers, every):
                po = {
                    qc: ps.tile(
                        [128, 1024], F32, tag="ps", name=f"po{b}_{hp}_{qc}"
                    )
                    for qc in range(2)
                }
                e = {0: scores_exp(b, hp, 0), 1: scores_exp(b, hp, 1)}
                for j in range(NQT):
                    if j + 2 < NQT:
                        e[j + 2] = scores_exp(b, hp, j + 2)
                    av(b, hp, j, e.pop(j), po)
                    if j == 3:
                        t_ = osp.tile(
                            [128, 1024], F16, tag=f"osm{hp}_0",
                            name=f"osm{b}_{hp}_0",
                        )
                        nc.vector.tensor_copy(t_, po[0])
                        osm[(b, hp, 0)] = t_
                    # fillers only in the second half of the j-loop, where
                    # po[0] is freed and PSUM slots are available
                    if j in (4, 6) and fillers:
                        fillers.pop(0)()
                t_ = osp.tile(
                    [128, 1024], F16, tag=f"osm{hp}_1", name=f"osm{b}_{hp}_1"
                )
                nc.vector.tensor_copy(t_, po[1])
                osm[(b, hp, 1)] = t_

            def norm_cluster(b):
                # recips run contiguously after every exp issued so far
                # (keeps ACT table loads at 2 per cluster); qc-major order
                # lets proj of this batch start after the qc0 muls
                last_exp = exp_all[-1]
                for qc in range(2):
                    for hp in range(4):
                        o_t = osm[(b, hp, qc)]
                        y0 = y0p.tile(
                            [64, 1024], F32, tag="y0", name=f"y0_{b}_{hp}_{qc}"
                        )
                        ri = _act_recip(nc, y0, o_t[64:128, :])
                        add_dep_helper(
                            ri.ins, last_exp, reason="recips after batch exps"
                        )
                        act_gate[0] = ri.ins
                        for hi in range(2):
                            base = 512 * hi
                            prow = 64 * hi
                            nc.vector.tensor_mul(
                                ots[b][hp][
                                    prow : prow + 64, 512 * qc : 512 * (qc + 1)
                                ],
                                o_t[0:64, base : base + 512],
                                y0[:, base : base + 512],
                            )
                        osm.pop((b, hp, qc))

            # ---- schedule ----
            for m in range(8):
                qkv_block(0, m)
            for st in range(NQT):
                v_block(0, st)

            fill1 = [
                (lambda m=m: qkv_block(1, m)) for m in range(8)
            ] + [(lambda st=st: v_block(1, st)) for st in range(NQT)]
            # interleave qkv and v blocks of batch 1
            fill1 = [fill1[i // 2 + 8 * (i % 2)] for i in range(16)]
            # 2 fillers per batch-0 pair (j=4,6); the remaining 8 bunch at
            # the batch boundary where the PE otherwise runs dry waiting
            # for the ACT engine to drain
            for hp in range(4):
                attn_pair(0, hp, fill1, 2)
            # the remaining fillers are PINNED (via dep edges on the last
            # batch-0 AV matmul) to the batch boundary, covering the PE
            # while ACT produces batch-1 pair-0 exps -- otherwise the
            # scheduler hoists them into earlier micro-gaps
            pe_gate[0] = last_av[0]
            while fill1:
                fill1.pop(0)()
            pe_gate[0] = None

            # batch-1 pair 0 runs UNGATED before cluster(0): its exps extend
            # the contiguous exp run, giving the PE AV work while the
            # reciprocal cluster drains on ACT
            attn_pair(1, 0, [], 2)
            norm_cluster(0)

            fill0 = [(lambda st=st: proj_block(0, st)) for st in range(NQT)]
            for hp in range(1, 4):
                attn_pair(1, hp, fill0, 2)
            pe_gate[0] = last_av[0]
            while fill0:
                fill0.pop(0)()
            pe_gate[0] = None
            norm_cluster(1)
            for st in range(NQT):
                proj_block(1, st)
    _split_multi_waits(nc)
    return nc


# ---------------------------------------------------------------------------
# Host orchestration
# ---------------------------------------------------------------------------

_CACHE = {}


def _get(name, builder):
    if name not in _CACHE:
        _CACHE[name] = builder()
    return _CACHE[name]


def _run_with_retry(nc, in_maps, trace=False, tries=3):
    import time as _time

    last = None
    for attempt in range(tries):
        try:
            return run_bass_kernel_spmd(
                nc, in_maps, core_ids=list(range(NCORES)), trace=trace
            )
        except Exception as e:  # transient NRT_EXEC_UNIT_UNRECOVERABLE etc.
            last = e
            _time.sleep(2.0 * (attempt + 1))
    raise last


def _silu(v):
    return v / (1.0 + np.exp(-v))


def kernel(
    time_embed,
    x,
    lin1_w,
    lin1_b,
    lin2_w,
    lin2_b,
    fW_attn_w,
    fW_attn_b,
    fb_attn,
    fW_proj_w,
    fW_proj_b,
    fb_proj,
    _trace=False,
    _times=None,
):
    f64 = np.float64
    # ---- host: time-embedding MLP ----
    t1 = _silu(time_embed.astype(f64) @ lin1_w.astype(f64) + lin1_b.astype(f64))
    t = t1 @ lin2_w.astype(f64) + lin2_b.astype(f64)   # [128]
    t16 = t.astype(np.float16)
    tcol = np.ascontiguousarray(t16[:, None])

    # ---- launch 1: W generation ----
    import ml_dtypes

    f8 = ml_dtypes.float8_e4m3
    QS_W, QS_T = 16.0, 32.0  # prescales keeping fp8 operands in normal range
    nc_gen = _get("gen", build_gen)
    fwa3 = fW_attn_w.reshape(TEMBED, E, J3)
    fq_all = (fwa3[:, :, :1024].astype(np.float32) * QS_W).astype(f8)
    fv_all = fwa3[:, :, 1024:].astype(np.float16)
    fwp_flat = fW_proj_w.reshape(TEMBED, E * E).astype(np.float16)
    tcol8 = np.ascontiguousarray((t * QS_T).astype(np.float32).astype(f8)[:, None])
    in_maps = []
    for c in range(NCORES):
        in_maps.append(
            {
                "tcol": tcol,
                "tcol8": tcol8,
                "fq": np.ascontiguousarray(
                    fq_all[:, 64 * c : 64 * (c + 1), :]
                ).reshape(TEMBED, EJ_Q),
                "fv": np.ascontiguousarray(
                    fv_all[:, 64 * c : 64 * (c + 1), :]
                ).reshape(TEMBED, EJ_V),
                "fwp": fwp_flat[:, EJ_P * c : EJ_P * (c + 1)],
            }
        )
    res1 = _run_with_retry(nc_gen, in_maps, trace=_trace)
    if _times is not None:
        _times.append(res1.exec_time_ns)

    # g*[p, i] = W[128*i + p]  ->  core slab flat = g*.T.ravel()
    Wq = np.concatenate(
        [res1.results[c]["gq"].T.ravel() for c in range(NCORES)]
    ).reshape(E, 1024) / (QS_W * QS_T)
    Wv = np.concatenate(
        [res1.results[c]["gv"].T.ravel() for c in range(NCORES)]
    ).reshape(E, 512)
    Wa = np.concatenate([Wq, Wv], axis=1)
    Wp = np.concatenate(
        [res1.results[c]["gp"].T.ravel() for c in range(NCORES)]
    ).reshape(E, E)
    Wa = Wa + fW_attn_b.reshape(E, J3)
    Wp = Wp + fW_proj_b.reshape(E, E)
    Wa[:, :512] *= 0.125  # fold 1/sqrt(D) into q columns

    # ---- host: biases ----
    b_attn = (t @ fb_attn.astype(f64).reshape(TEMBED, J3)).astype(np.float32)
    bqk_host = b_attn[:1024].copy()
    bqk_host[:512] *= 0.125
    bqk_in = np.ascontiguousarray(bqk_host.reshape(8, 128).T)
    b_v = b_attn[1024:]
    b_proj = (t @ fb_proj.astype(f64)).astype(np.float32)
    brow = (b_v.astype(f64) @ Wp.astype(f64) + b_proj).astype(np.float16)
    brow_in = np.ascontiguousarray(brow[None, :])
    mask_in = np.triu(np.ones((128, 128), dtype=np.float16))
    ones_in = np.ones((128, 128), dtype=np.float16)
    Wa16 = Wa.astype(np.float16)
    Wp16 = Wp.astype(np.float16)

    # ---- launch 2: attention ----
    nc_attn = _get("attn", build_attn)
    in_maps = []
    for c in range(NCORES):
        xt_c = np.ascontiguousarray(
            x[BPC * c : BPC * (c + 1)].reshape(S2, E).T
        )
        in_maps.append(
            {
                "xt": xt_c.astype(np.float16),
                "wa": Wa16,
                "wp": Wp16,
                "bqk": bqk_in,
                "brow": brow_in,
                "mask": mask_in,
                "ones": ones_in,
            }
        )
    res2 = _run_with_retry(nc_attn, in_maps, trace=_trace)
    if _times is not None:
        _times.append(res2.exec_time_ns)

    out = np.empty((B, S, E), dtype=np.float32)
    for c in range(NCORES):
        out[BPC * c : BPC * (c + 1)] = res2.results[c]["out"].reshape(BPC, S, E)
    return out
